# revision 8
# baseline (speedup 1.0000x reference)
"""Trainium2 Bass kernel for nn_Drnet (histogram-binned multi-head MLP).

Contract: kernel(**inputs) takes the FULL unsharded inputs (t [N], x [N,100],
trunk + 5-head weights) and returns the FULL [N, 1] float32 output.

Strategy (v2):
  * Host: bin rows by floor(t*5) exactly as the reference, stable-sort by
    bin, shard contiguously across 8 cores, pad each per-core bin segment to
    a 512-row tile so every tile is single-bin. Per-tile head weights are
    shipped as data, so one SPMD program serves all cores.
  * The treatment t and a constant-1 channel ride through every layer as
    extra hidden channels (identity weight columns), so each Treat_Linear
    layer (feat@W + t*tw + b) is ONE matmul — the bias rides on the ones
    channel, making every PSUM->SBUF evacuation a uniform bias-free relu.
  * All matmul fmaps use >=96 partitions (hidden tiles are padded with
    zero rows 66:96): measured TRN2 behaviour is ~2x row rate for K>=96
    (512-row matmul ~230ns vs ~450ns at K<=80).
  * Per pair of tiles (1024 rows) and per layer, ONE [*,1024] PSUM tile
    (2 banks) receives both tiles' matmuls and is evacuated by a single
    ACT/DVE op. A 4-buffer rotation of [97,1024] PSUM tiles fills all 8
    banks. The head-L3 row (M=1) lands at partition 96 of the same PSUM
    tile as head-L2, so no separate output bank is needed.
  * Input x is streamed bf16 with t as channel 100; each group DMA is
    split across the SP and Activation HWDGE queues (single-queue DMA
    streams measure ~40GB/s/core; split across queues ~3-4x that).
"""
import numpy as np
import ml_dtypes

import concourse.bass as bass
import concourse.tile as tile
from concourse import mybir
from concourse.bass_utils import run_bass_kernel_spmd
from concourse.vector_clock import ScopedClock
from contextlib import ExitStack

BF16 = ml_dtypes.bfloat16

NCORES = 8
N = 1_000_000
D = 100
H = 64
NH = 5
TILE = 512
RPC = N // NCORES            # 125000 rows per core
NT = 252                     # tiles per core (>= ceil((RPC + 5*511)/512), mult of 4)
RPAD = NT * TILE             # 129024 padded rows per core
NP = NT // 2                 # 126 pairs per core
NG = NT // 4                 # 63 groups (4 tiles) per core, for input DMA
GT = 4 * TILE                # 2048 rows per group
PT = 2 * TILE                # 1024 rows per pair

KX = 101                     # input channels: 100 features + t at row 100
HA = 66                      # augmented hidden: 64 + t@64 + ones@65
KP = 96                      # padded fmap partitions (K>=96 => full PE rate)
WTC = 2 * HA + 1             # per-tile head weight block cols: H1|H2|H3 = 133
WGC = 2 * WTC                # per-pair weight cols
ORow = 96                    # psum partition carrying the head-L3 output

_FP32 = mybir.dt.float32
_BF16 = mybir.dt.bfloat16


_MAX_WAITS = 1
# This walrus build allows only ONE embedded sync wait on every instruction
# type tested — keep the global limit at 1 and spill all excess waits onto
# same-engine nops.
_MAX_WAITS_BY_TYPE = {}
_DEFAULT_MAX_WAITS = 1


class _SplitDrainTileContext(tile.TileContext):
    """Workaround: this walrus build rejects >1 embedded sync waits per
    instruction. Excess waits are moved onto same-engine nops inserted
    immediately before the overloaded instruction (same semantics: the
    engine's sequencer satisfies them in program order). The kernel-tail
    Drain additionally gets its waits via a chain of SP nops."""

    def _split_excess_waits(self):
        nc = self.nc
        for f in nc.m.functions:
            for bb in f.blocks:
                new_list = []
                changed = False
                for inst in bb.instructions:
                    si = inst.sync_info
                    waits = list(si.on_wait) if si and si.on_wait else []
                    maxw = _MAX_WAITS_BY_TYPE.get(
                        type(inst).__name__, _DEFAULT_MAX_WAITS)
                    if len(waits) > maxw:
                        changed = True
                        excess, keep = waits[:-maxw], waits[-maxw:]
                        for i in range(0, len(excess), _DEFAULT_MAX_WAITS):
                            nop = mybir.InstNoOp(
                                name=nc.get_next_instruction_name(),
                                ins=[], outs=[])
                            nop.engine = inst.engine
                            nop.sync_info = mybir.SyncInfo(
                                on_wait=list(excess[i:i + _DEFAULT_MAX_WAITS]),
                                on_update=[])
                            nc.register_instruction(nop)
                            new_list.append(nop)
                        inst.sync_info = mybir.SyncInfo(
                            on_wait=keep,
                            on_update=list(si.on_update) if si.on_update else [])
                    new_list.append(inst)
                if changed:
                    bb.instructions[:] = new_list

    def _drain_and_barrier(self, tick_clock, wait_clock):
        gc = tick_clock.global_clock
        needs = []
        for scope, vc in ScopedClock({None: gc}).items():
            for proc in range(len(vc)):
                t = vc[proc]
                if t > 0:
                    needs.append((scope, proc, t))
        for scope, proc, t in needs:
            nop = self.nc.sync.nop()
            partial = ScopedClock()
            partial.require_at_least(scope, proc, t)
            wait_clock.add_sem_waits(nop.ins, partial)
        self.nc.sync.drain()
        self.nc.all_engine_barrier()
        assert self.sems is not None
        popped = self.nc._tile_sem_poison_stack.pop()
        assert popped is self._sem_poison
        self.nc.clear_and_free_semaphores(list(self.sems.allocated().values()))
        self.nc.all_engine_barrier()
        self._split_excess_waits()


def _build_program(loop_n=1):
    nc = bass.Bass()
    xt_h = nc.dram_tensor("xt", [NG, KX, GT], _BF16, kind="ExternalInput")
    wg_h = nc.dram_tensor("wg", [NG, HA, 2 * WGC], _BF16, kind="ExternalInput")
    w1_h = nc.dram_tensor("w1", [KX, HA], _BF16, kind="ExternalInput")
    w2_h = nc.dram_tensor("w2", [KP, HA], _BF16, kind="ExternalInput")
    b1_h = nc.dram_tensor("b1", [HA, 1], _FP32, kind="ExternalInput")
    out_h = nc.dram_tensor("out", [NT * TILE], _FP32, kind="ExternalOutput")

    RELU = mybir.ActivationFunctionType.Relu
    IDENT = mybir.ActivationFunctionType.Identity
    ADD = mybir.AluOpType.add
    MAX = mybir.AluOpType.max

    with _SplitDrainTileContext(nc) as tc, ExitStack() as ctx:
        statics = ctx.enter_context(tc.tile_pool(name="statics", bufs=1))
        xpool = ctx.enter_context(tc.tile_pool(name="x", bufs=4))
        wpool = ctx.enter_context(tc.tile_pool(name="w", bufs=8))
        hpool = ctx.enter_context(tc.tile_pool(name="h", bufs=10))
        opool = ctx.enter_context(tc.tile_pool(name="o", bufs=3))
        pspool = ctx.enter_context(tc.tile_pool(name="ps", bufs=3, space="PSUM"))
        ps5pool = ctx.enter_context(tc.tile_pool(name="ps5", bufs=2, space="PSUM"))

        w1_sb = statics.tile([KX, HA], _BF16)
        nc.sync.dma_start(out=w1_sb, in_=w1_h[:, :])
        w2_sb = statics.tile([KP, HA], _BF16)
        nc.sync.dma_start(out=w2_sb, in_=w2_h[:, :])
        b1_sb = statics.tile([HA, 1], _FP32)
        nc.sync.dma_start(out=b1_sb, in_=b1_h[:, :])
        zb_sb = statics.tile([128, 1], _FP32)
        nc.vector.memset(zb_sb, 0.0)

        # pre-zero the pad rows (66:96) of the rotating weight and hidden
        # buffers once; later DMAs/evacs only touch rows 0:66, so the pad
        # rows stay zero for every rotation (and in every For_i iteration).
        for i in range(8):
            wb = wpool.tile([KP, 2 * WGC], _BF16, tag="wt")
            nc.vector.memset(wb[H:KP, :], 0.0)
        for i in range(10):
            hb = hpool.tile([KP, PT], _BF16, tag="h")
            nc.vector.memset(hb[H:KP, :], 0.0)

        ST = {}   # pair -> state
        GS = {}   # group -> state (xg shared by 2 pairs)

        def LOAD(p):
            g, pi = divmod(p, 2)
            s = {"g": g, "pi": pi}
            if pi == 0:
                xg = xpool.tile([KX, GT], _BF16, tag="xg")
                nc.sync.dma_start(out=xg[0:51, :], in_=xt_h[g, 0:51, :])
                nc.scalar.dma_start(out=xg[51:KX, :], in_=xt_h[g, 51:KX, :])
                wt = wpool.tile([KP, 2 * WGC], _BF16, tag="wt")
                if g % 2 == 0:
                    nc.sync.dma_start(out=wt[0:HA, :], in_=wg_h[g, :, :])
                else:
                    nc.scalar.dma_start(out=wt[0:HA, :], in_=wg_h[g, :, :])
                GS[g] = {"xg": xg, "wt": wt}
            ST[p] = s

        def T1(p):
            s = ST[p]
            s.update(GS[s["g"]])
            s["wc"] = WGC * s["pi"]
            off = PT * s["pi"]
            xg = s["xg"]
            ps = pspool.tile([ORow + 1, PT], _FP32, tag="ps", name=f"ps1_{p}")
            nc.tensor.matmul(ps[0:HA, 0:TILE], w1_sb, xg[:, off:off + TILE],
                             start=True, stop=True)
            nc.tensor.matmul(ps[0:HA, TILE:PT], w1_sb,
                             xg[:, off + TILE:off + PT],
                             start=True, stop=True)
            s["ps1"] = ps

        def A1(p):
            s = ST[p]
            h1 = hpool.tile([KP, PT], _BF16, tag="h")
            nc.scalar.activation(h1[0:HA, :], s.pop("ps1")[0:HA, :], RELU,
                                 bias=b1_sb)
            s["h1"] = h1

        def T2(p):
            s = ST[p]
            h1 = s.pop("h1")
            ps = pspool.tile([ORow + 1, PT], _FP32, tag="ps", name=f"ps2_{p}")
            nc.tensor.matmul(ps[0:HA, 0:TILE], w2_sb, h1[0:KP, 0:TILE],
                             start=True, stop=True)
            nc.tensor.matmul(ps[0:HA, TILE:PT], w2_sb, h1[0:KP, TILE:PT],
                             start=True, stop=True)
            s["ps2"] = ps

        def A2(p):
            s = ST[p]
            h2 = hpool.tile([KP, PT], _BF16, tag="h")
            nc.vector.tensor_scalar(out=h2[0:HA, :], in0=s.pop("ps2")[0:HA, :],
                                    scalar1=0.0, scalar2=None, op0=MAX)
            s["h2"] = h2

        def HL1(p):
            s = ST[p]
            h2, wt = s.pop("h2"), s["wt"]
            ps = pspool.tile([ORow + 1, PT], _FP32, tag="ps", name=f"ps3_{p}")
            wc = s["wc"]
            nc.tensor.matmul(ps[0:HA, 0:TILE], wt[:, wc:wc + HA],
                             h2[0:KP, 0:TILE], start=True, stop=True)
            nc.tensor.matmul(ps[0:HA, TILE:PT], wt[:, wc + WTC:wc + WTC + HA],
                             h2[0:KP, TILE:PT], start=True, stop=True)
            s["ps3"] = ps

        def A3(p):
            s = ST[p]
            a1 = hpool.tile([KP, PT], _BF16, tag="h")
            nc.vector.tensor_scalar(out=a1[0:HA, :], in0=s.pop("ps3")[0:HA, :],
                                    scalar1=0.0, scalar2=None, op0=MAX)
            s["a1"] = a1

        def HL2(p):
            s = ST[p]
            a1, wt = s.pop("a1"), s["wt"]
            ps = pspool.tile([ORow + 1, PT], _FP32, tag="ps", name=f"ps4_{p}")
            wc = s["wc"]
            nc.tensor.matmul(ps[0:HA, 0:TILE], wt[:, wc + HA:wc + 2 * HA],
                             a1[0:KP, 0:TILE], start=True, stop=True)
            nc.tensor.matmul(ps[0:HA, TILE:PT],
                             wt[:, wc + WTC + HA:wc + WTC + 2 * HA],
                             a1[0:KP, TILE:PT], start=True, stop=True)
            s["ps4"] = ps

        def A4(p):
            s = ST[p]
            a2 = hpool.tile([KP, PT], _BF16, tag="h")
            nc.scalar.activation(a2[0:HA, :], s.pop("ps4")[0:HA, :], RELU,
                                 bias=zb_sb[0:HA, :])
            s["a2"] = a2

        def HL3(p):
            # head L3 rows accumulate in a per-group [97,512] bank at
            # partitions 32*tile_in_group; evacuated once per group.
            s = ST[p]
            g, pi = s["g"], s["pi"]
            a2, wt = s.pop("a2"), s["wt"]
            if pi == 0:
                GS[g]["ps5"] = ps5pool.tile([ORow + 1, TILE], _FP32,
                                            tag="ps5", name=f"ps5_{g}")
            ps5 = GS[g]["ps5"]
            c = 64 * pi
            wc = s["wc"]
            nc.tensor.matmul(ps5[c:c + 1, :], wt[:, wc + 2 * HA:wc + WTC],
                             a2[0:KP, 0:TILE],
                             start=True, stop=True, tile_position=(0, c))
            nc.tensor.matmul(ps5[c + 32:c + 33, :],
                             wt[:, wc + WTC + 2 * HA:wc + WGC],
                             a2[0:KP, TILE:PT],
                             start=True, stop=True, tile_position=(0, c + 32))

        def OB(p):
            s = ST.pop(p)
            if s["pi"] != 1:
                return
            g = s["g"]
            gs = GS.pop(g)
            ps5 = gs["ps5"]
            ob = opool.tile([ORow + 1, TILE], _FP32, tag="ob")
            if g % 2 == 0:
                nc.scalar.activation(ob, ps5, IDENT, bias=zb_sb[0:97, :])
            else:
                nc.vector.tensor_scalar(out=ob, in0=ps5,
                                        scalar1=0.0, scalar2=None, op0=ADD)
            dst = out_h[4 * g * TILE:4 * g * TILE + 4 * TILE]
            if g % 2 == 0:
                nc.sync.dma_start(out=dst, in_=ob[0:ORow + 1:32, :])
            else:
                nc.scalar.dma_start(out=dst, in_=ob[0:ORow + 1:32, :])

        # (offset, fn, pmax) in within-step emission order. Offsets place
        # each evacuation in the SAME step as its PSUM producer (emitted
        # after it), so every buffer-reuse wait points at the PREVIOUS
        # step. Engine op order pairs each evac with an early PE producer:
        # PE runs H2, T2, T1, H1, HL3; scalar evacs {OB, A4, A1}; DVE
        # evacs {A2, A3}.
        STAGES = [(0, LOAD, NP), (13, OB, NP), (10, HL2, NP), (10, A4, NP),
                  (6, T2, NP), (6, A2, NP), (4, T1, NP), (4, A1, NP),
                  (8, HL1, NP), (8, A3, NP), (11, HL3, NP)]
        NSTEP = max(k for k, _, _ in STAGES) + 1

        def emit_body():
            for v in range(NP + NSTEP - 1):
                for k, fn, pmax in STAGES:
                    p = v - k
                    if 0 <= p < pmax:
                        fn(p)

        if loop_n == 1:
            emit_body()
        else:
            with tc.For_i(0, loop_n, 1):
                emit_body()
    return nc


_PROGRAM = None
last_results = None


def _get_program():
    global _PROGRAM
    if _PROGRAM is None:
        _PROGRAM = _build_program()
    return _PROGRAM


def make_in_maps(t, x, dW1, db1, dW2, db2,
                 hw1, htw1, hb1, hw2, htw2, hb2, hw3, htw3, hb3):
    """Host-side sharding/packing. Returns (in_maps, lidx_all, order)."""
    t = np.asarray(t, np.float32)
    x = np.asarray(x, np.float32)

    # --- bin + stable sort (binning identical to the reference) ---
    bins = np.clip(np.floor(t * np.float32(NH)).astype(np.int32), 0, NH - 1)
    order = np.argsort(bins, kind="stable")
    t_s = t[order]
    x_s = x[order]
    bins_s = bins[order]

    # --- static trunk weights with t/ones identity channels ---
    w1a = np.zeros((KX, HA), np.float32)
    w1a[0:D, 0:H] = dW1
    w1a[D, H] = 1.0                      # t passthrough
    b1a = np.zeros((HA, 1), np.float32)
    b1a[0:H, 0] = db1
    b1a[H + 1, 0] = 1.0                  # ones channel born from the bias
    w2a = np.zeros((KP, HA), np.float32)
    w2a[0:H, 0:H] = dW2
    w2a[H, H] = 1.0                      # t passthrough
    w2a[H + 1, 0:H] = db2                # bias via ones channel
    w2a[H + 1, H + 1] = 1.0              # ones passthrough

    # --- per-bin head weight blocks [HA, WTC] = H1(66) | H2(66) | H3(1) ---
    WQ = np.zeros((NH, HA, WTC), np.float32)
    for q in range(NH):
        for li, (hw, htw, hb) in enumerate(
                ((hw1, htw1, hb1), (hw2, htw2, hb2))):
            c = HA * li
            WQ[q, 0:H, c:c + H] = hw[q]
            WQ[q, H, c:c + H] = htw[q]
            WQ[q, H + 1, c:c + H] = hb[q]
            WQ[q, H, c + H] = 1.0        # t passthrough
            WQ[q, H + 1, c + H + 1] = 1.0  # ones passthrough
        WQ[q, 0:H, 2 * HA] = hw3[q][:, 0]
        WQ[q, H, 2 * HA] = htw3[q, 0]
        WQ[q, H + 1, 2 * HA] = hb3[q, 0]

    # --- per-core padded tiling (each 512-row tile single-bin) ---
    in_maps = []
    lidx_all = []
    for c in range(NCORES):
        s = c * RPC
        tb = bins_s[s:s + RPC]
        parts = []
        tile_bins = []
        for q in range(NH):
            sel = np.nonzero(tb == q)[0].astype(np.int64)
            if len(sel) == 0:
                continue
            npad = (-len(sel)) % TILE
            parts.append(np.concatenate([sel, np.full(npad, -1, np.int64)]))
            tile_bins += [q] * ((len(sel) + npad) // TILE)
        lidx = np.concatenate(parts)
        rem = RPAD - len(lidx)
        assert rem >= 0 and rem % TILE == 0
        lidx = np.concatenate([lidx, np.full(rem, -1, np.int64)])
        tile_bins += [0] * (rem // TILE)
        tile_bins = np.asarray(tile_bins, np.int64)
        lidx_all.append(lidx)

        safe = np.where(lidx >= 0, lidx, 0)
        feat = x_s[s:s + RPC][safe]
        tval = t_s[s:s + RPC][safe]
        feat[lidx < 0] = 0.0
        tval[lidx < 0] = 0.0
        xt = np.empty((NG, KX, GT), np.float32)
        xt[:, 0:D, :] = feat.reshape(NG, GT, D).transpose(0, 2, 1)
        xt[:, D, :] = tval.reshape(NG, GT)

        wg = WQ[tile_bins].reshape(NG, 4, HA, WTC).transpose(
            0, 2, 1, 3).reshape(NG, HA, 2 * WGC)

        in_maps.append({
            "xt": xt.astype(BF16), "wg": np.ascontiguousarray(wg).astype(BF16),
            "w1": w1a.astype(BF16), "w2": w2a.astype(BF16), "b1": b1a,
        })
    return in_maps, lidx_all, order


def postprocess(core_outs, lidx_all, order):
    """core_outs: list of per-core 'out' arrays [NT, TILE] -> full [N, 1]."""
    out_sorted = np.empty(N, np.float32)
    for c in range(NCORES):
        flat = np.asarray(core_outs[c], np.float32).reshape(RPAD)
        lidx = lidx_all[c]
        valid = lidx >= 0
        seg = np.empty(RPC, np.float32)
        seg[lidx[valid]] = flat[valid]
        out_sorted[c * RPC:(c + 1) * RPC] = seg
    out = np.empty(N, np.float32)
    out[order] = out_sorted
    return out[:, None]


def kernel(t, x, dW1, db1, dW2, db2,
           hw1, htw1, hb1, hw2, htw2, hb2, hw3, htw3, hb3):
    in_maps, lidx_all, order = make_in_maps(
        t, x, dW1, db1, dW2, db2,
        hw1, htw1, hb1, hw2, htw2, hb2, hw3, htw3, hb3)
    nc = _get_program()
    res = run_bass_kernel_spmd(nc, in_maps, list(range(NCORES)))
    global last_results
    last_results = res
    return postprocess([res.results[c]["out"] for c in range(NCORES)],
                       lidx_all, order)


# revision 9
# speedup vs baseline: 1.3089x; 1.3089x over previous
"""Trainium2 Bass kernel for nn_Drnet (histogram-binned multi-head MLP).

Contract: kernel(**inputs) takes the FULL unsharded inputs (t [N], x [N,100],
trunk + 5-head weights) and returns the FULL [N, 1] float32 output.

Strategy (v2):
  * Host: bin rows by floor(t*5) exactly as the reference, stable-sort by
    bin, shard contiguously across 8 cores, pad each per-core bin segment to
    a 512-row tile so every tile is single-bin. Per-tile head weights are
    shipped as data, so one SPMD program serves all cores.
  * The treatment t and a constant-1 channel ride through every layer as
    extra hidden channels (identity weight columns), so each Treat_Linear
    layer (feat@W + t*tw + b) is ONE matmul — the bias rides on the ones
    channel, making every PSUM->SBUF evacuation a uniform bias-free relu.
  * All matmul fmaps use >=96 partitions (hidden tiles are padded with
    zero rows 66:96): measured TRN2 behaviour is ~2x row rate for K>=96
    (512-row matmul ~230ns vs ~450ns at K<=80).
  * Per pair of tiles (1024 rows) and per layer, ONE [*,1024] PSUM tile
    (2 banks) receives both tiles' matmuls and is evacuated by a single
    ACT/DVE op. A 4-buffer rotation of [97,1024] PSUM tiles fills all 8
    banks. The head-L3 row (M=1) lands at partition 96 of the same PSUM
    tile as head-L2, so no separate output bank is needed.
  * Input x is streamed bf16 with t as channel 100; each group DMA is
    split across the SP and Activation HWDGE queues (single-queue DMA
    streams measure ~40GB/s/core; split across queues ~3-4x that).
"""
import numpy as np
import ml_dtypes

import concourse.bass as bass
import concourse.tile as tile
from concourse import mybir
from concourse.bass_utils import run_bass_kernel_spmd
from concourse.vector_clock import ScopedClock
from contextlib import ExitStack

BF16 = ml_dtypes.bfloat16

NCORES = 8
N = 1_000_000
D = 100
H = 64
NH = 5
TILE = 512
RPC = N // NCORES            # 125000 rows per core
NT = 252                     # tiles per core (>= ceil((RPC + 5*511)/512), mult of 4)
RPAD = NT * TILE             # 129024 padded rows per core
NP = NT // 2                 # 126 pairs per core
NG = NT // 4                 # 63 groups (4 tiles) per core, for input DMA
GT = 4 * TILE                # 2048 rows per group
PT = 2 * TILE                # 1024 rows per pair

KX = 101                     # input channels: 100 features + t at row 100
HA = 66                      # augmented hidden: 64 + t@64 + ones@65
KP = 96                      # padded fmap partitions (K>=96 => full PE rate)
WTC = 2 * HA + 1             # per-tile head weight block cols: H1|H2|H3 = 133
WGC = 2 * WTC                # per-pair weight cols
ORow = 96                    # psum partition carrying the head-L3 output

_FP32 = mybir.dt.float32
_BF16 = mybir.dt.bfloat16


_MAX_WAITS = 1
# This walrus build allows only ONE embedded sync wait on every instruction
# type tested — keep the global limit at 1 and spill all excess waits onto
# same-engine nops.
_MAX_WAITS_BY_TYPE = {}
_DEFAULT_MAX_WAITS = 1


class _SplitDrainTileContext(tile.TileContext):
    """Workaround: this walrus build rejects >1 embedded sync waits per
    instruction. Excess waits are moved onto same-engine nops inserted
    immediately before the overloaded instruction (same semantics: the
    engine's sequencer satisfies them in program order). The kernel-tail
    Drain additionally gets its waits via a chain of SP nops."""

    def _split_excess_waits(self):
        nc = self.nc
        for f in nc.m.functions:
            for bb in f.blocks:
                new_list = []
                changed = False
                for inst in bb.instructions:
                    si = inst.sync_info
                    waits = list(si.on_wait) if si and si.on_wait else []
                    maxw = _MAX_WAITS_BY_TYPE.get(
                        type(inst).__name__, _DEFAULT_MAX_WAITS)
                    if len(waits) > maxw:
                        changed = True
                        excess, keep = waits[:-maxw], waits[-maxw:]
                        for i in range(0, len(excess), _DEFAULT_MAX_WAITS):
                            nop = mybir.InstNoOp(
                                name=nc.get_next_instruction_name(),
                                ins=[], outs=[])
                            nop.engine = inst.engine
                            nop.sync_info = mybir.SyncInfo(
                                on_wait=list(excess[i:i + _DEFAULT_MAX_WAITS]),
                                on_update=[])
                            nc.register_instruction(nop)
                            new_list.append(nop)
                        inst.sync_info = mybir.SyncInfo(
                            on_wait=keep,
                            on_update=list(si.on_update) if si.on_update else [])
                    new_list.append(inst)
                if changed:
                    bb.instructions[:] = new_list

    def _drain_and_barrier(self, tick_clock, wait_clock):
        gc = tick_clock.global_clock
        needs = []
        for scope, vc in ScopedClock({None: gc}).items():
            for proc in range(len(vc)):
                t = vc[proc]
                if t > 0:
                    needs.append((scope, proc, t))
        for scope, proc, t in needs:
            nop = self.nc.sync.nop()
            partial = ScopedClock()
            partial.require_at_least(scope, proc, t)
            wait_clock.add_sem_waits(nop.ins, partial)
        self.nc.sync.drain()
        self.nc.all_engine_barrier()
        assert self.sems is not None
        popped = self.nc._tile_sem_poison_stack.pop()
        assert popped is self._sem_poison
        self.nc.clear_and_free_semaphores(list(self.sems.allocated().values()))
        self.nc.all_engine_barrier()
        self._split_excess_waits()


def _build_program(loop_n=1):
    nc = bass.Bass()
    xt_h = nc.dram_tensor("xt", [NG, KX, GT], _BF16, kind="ExternalInput")
    wg_h = nc.dram_tensor("wg", [NG, HA, 2 * WGC], _BF16, kind="ExternalInput")
    w1_h = nc.dram_tensor("w1", [KX, HA], _BF16, kind="ExternalInput")
    w2_h = nc.dram_tensor("w2", [KP, HA], _BF16, kind="ExternalInput")
    b1_h = nc.dram_tensor("b1", [HA, 1], _FP32, kind="ExternalInput")
    out_h = nc.dram_tensor("out", [NT * TILE], _FP32, kind="ExternalOutput")

    RELU = mybir.ActivationFunctionType.Relu
    IDENT = mybir.ActivationFunctionType.Identity
    ADD = mybir.AluOpType.add
    MAX = mybir.AluOpType.max

    with _SplitDrainTileContext(nc) as tc, ExitStack() as ctx:
        statics = ctx.enter_context(tc.tile_pool(name="statics", bufs=1))
        xpool = ctx.enter_context(tc.tile_pool(name="x", bufs=4))
        wpool = ctx.enter_context(tc.tile_pool(name="w", bufs=8))
        hpool = ctx.enter_context(tc.tile_pool(name="h", bufs=10))
        opool = ctx.enter_context(tc.tile_pool(name="o", bufs=3))
        pspool = ctx.enter_context(tc.tile_pool(name="ps", bufs=3, space="PSUM"))
        ps5pool = ctx.enter_context(tc.tile_pool(name="ps5", bufs=2, space="PSUM"))

        w1_sb = statics.tile([KX, HA], _BF16)
        nc.sync.dma_start(out=w1_sb, in_=w1_h[:, :])
        w2_sb = statics.tile([KP, HA], _BF16)
        nc.sync.dma_start(out=w2_sb, in_=w2_h[:, :])
        b1_sb = statics.tile([HA, 1], _FP32)
        nc.sync.dma_start(out=b1_sb, in_=b1_h[:, :])
        zb_sb = statics.tile([128, 1], _FP32)
        nc.vector.memset(zb_sb, 0.0)

        # pre-zero the pad rows (66:96) of the rotating weight and hidden
        # buffers once; later DMAs/evacs only touch rows 0:66, so the pad
        # rows stay zero for every rotation (and in every For_i iteration).
        for i in range(8):
            wb = wpool.tile([KP, 2 * WGC], _BF16, tag="wt")
            nc.vector.memset(wb[H:KP, :], 0.0)
        for i in range(10):
            hb = hpool.tile([KP, PT], _BF16, tag="h")
            nc.vector.memset(hb[H:KP, :], 0.0)

        ST = {}   # pair -> state
        GS = {}   # group -> state (xg shared by 2 pairs)

        def LOAD(p):
            g, pi = divmod(p, 2)
            s = {"g": g, "pi": pi}
            if pi == 0:
                xg = xpool.tile([KX, GT], _BF16, tag="xg")
                nc.sync.dma_start(out=xg[0:51, :], in_=xt_h[g, 0:51, :])
                nc.scalar.dma_start(out=xg[51:KX, :], in_=xt_h[g, 51:KX, :])
                wt = wpool.tile([KP, 2 * WGC], _BF16, tag="wt")
                if g % 2 == 0:
                    nc.sync.dma_start(out=wt[0:HA, :], in_=wg_h[g, :, :])
                else:
                    nc.scalar.dma_start(out=wt[0:HA, :], in_=wg_h[g, :, :])
                GS[g] = {"xg": xg, "wt": wt}
            ST[p] = s

        def T1(p):
            s = ST[p]
            s.update(GS[s["g"]])
            s["wc"] = WGC * s["pi"]
            off = PT * s["pi"]
            xg = s["xg"]
            ps = pspool.tile([ORow + 1, PT], _FP32, tag="ps", name=f"ps1_{p}")
            nc.tensor.matmul(ps[0:HA, 0:TILE], w1_sb, xg[:, off:off + TILE],
                             start=True, stop=True)
            nc.tensor.matmul(ps[0:HA, TILE:PT], w1_sb,
                             xg[:, off + TILE:off + PT],
                             start=True, stop=True)
            s["ps1"] = ps

        def A1(p):
            s = ST[p]
            h1 = hpool.tile([KP, PT], _BF16, tag="h")
            nc.scalar.activation(h1[0:HA, :], s.pop("ps1")[0:HA, :], RELU,
                                 bias=b1_sb)
            s["h1"] = h1

        def T2(p):
            s = ST[p]
            h1 = s.pop("h1")
            ps = pspool.tile([ORow + 1, PT], _FP32, tag="ps", name=f"ps2_{p}")
            nc.tensor.matmul(ps[0:HA, 0:TILE], w2_sb, h1[0:KP, 0:TILE],
                             start=True, stop=True)
            nc.tensor.matmul(ps[0:HA, TILE:PT], w2_sb, h1[0:KP, TILE:PT],
                             start=True, stop=True)
            s["ps2"] = ps

        def A2(p):
            s = ST[p]
            h2 = hpool.tile([KP, PT], _BF16, tag="h")
            nc.vector.tensor_scalar(out=h2[0:HA, :], in0=s.pop("ps2")[0:HA, :],
                                    scalar1=0.0, scalar2=None, op0=MAX)
            s["h2"] = h2

        def HL1(p):
            s = ST[p]
            h2, wt = s.pop("h2"), s["wt"]
            ps = pspool.tile([ORow + 1, PT], _FP32, tag="ps", name=f"ps3_{p}")
            wc = s["wc"]
            nc.tensor.matmul(ps[0:HA, 0:TILE], wt[:, wc:wc + HA],
                             h2[0:KP, 0:TILE], start=True, stop=True)
            nc.tensor.matmul(ps[0:HA, TILE:PT], wt[:, wc + WTC:wc + WTC + HA],
                             h2[0:KP, TILE:PT], start=True, stop=True)
            s["ps3"] = ps

        def A3(p):
            s = ST[p]
            a1 = hpool.tile([KP, PT], _BF16, tag="h")
            nc.vector.tensor_scalar(out=a1[0:HA, :], in0=s.pop("ps3")[0:HA, :],
                                    scalar1=0.0, scalar2=None, op0=MAX)
            s["a1"] = a1

        def HL2(p):
            s = ST[p]
            a1, wt = s.pop("a1"), s["wt"]
            ps = pspool.tile([ORow + 1, PT], _FP32, tag="ps", name=f"ps4_{p}")
            wc = s["wc"]
            nc.tensor.matmul(ps[0:HA, 0:TILE], wt[:, wc + HA:wc + 2 * HA],
                             a1[0:KP, 0:TILE], start=True, stop=True)
            nc.tensor.matmul(ps[0:HA, TILE:PT],
                             wt[:, wc + WTC + HA:wc + WTC + 2 * HA],
                             a1[0:KP, TILE:PT], start=True, stop=True)
            s["ps4"] = ps

        def A4(p):
            s = ST[p]
            a2 = hpool.tile([KP, PT], _BF16, tag="h")
            nc.scalar.activation(a2[0:HA, :], s.pop("ps4")[0:HA, :], RELU,
                                 bias=zb_sb[0:HA, :])
            s["a2"] = a2

        def HL3(p):
            # head L3 rows accumulate in a per-group [97,512] bank at
            # partitions 32*tile_in_group; evacuated once per group.
            s = ST[p]
            g, pi = s["g"], s["pi"]
            a2, wt = s.pop("a2"), s["wt"]
            if pi == 0:
                GS[g]["ps5"] = ps5pool.tile([ORow + 1, TILE], _FP32,
                                            tag="ps5", name=f"ps5_{g}")
            ps5 = GS[g]["ps5"]
            c = 64 * pi
            wc = s["wc"]
            nc.tensor.matmul(ps5[c:c + 1, :], wt[:, wc + 2 * HA:wc + WTC],
                             a2[0:KP, 0:TILE],
                             start=True, stop=True, tile_position=(0, c))
            nc.tensor.matmul(ps5[c + 32:c + 33, :],
                             wt[:, wc + WTC + 2 * HA:wc + WGC],
                             a2[0:KP, TILE:PT],
                             start=True, stop=True, tile_position=(0, c + 32))

        def OB(p):
            s = ST.pop(p)
            if s["pi"] != 1:
                return
            g = s["g"]
            gs = GS.pop(g)
            ps5 = gs["ps5"]
            ob = opool.tile([ORow + 1, TILE], _FP32, tag="ob")
            if g % 2 == 0:
                nc.scalar.activation(ob, ps5, IDENT, bias=zb_sb[0:97, :])
            else:
                nc.vector.tensor_scalar(out=ob, in0=ps5,
                                        scalar1=0.0, scalar2=None, op0=ADD)
            dst = out_h[4 * g * TILE:4 * g * TILE + 4 * TILE]
            if g % 2 == 0:
                nc.sync.dma_start(out=dst, in_=ob[0:ORow + 1:32, :])
            else:
                nc.scalar.dma_start(out=dst, in_=ob[0:ORow + 1:32, :])

        # (offset, fn, pmax) in within-step emission order. Offsets place
        # each evacuation in the SAME step as its PSUM producer (emitted
        # after it), so every buffer-reuse wait points at the PREVIOUS
        # step. Engine op order pairs each evac with an early PE producer:
        # PE runs H2, T2, T1, H1, HL3; scalar evacs {OB, A4, A1}; DVE
        # evacs {A2, A3}.
        STAGES = [(0, LOAD, NP), (11, OB, NP), (8, HL2, NP), (8, A4, NP),
                  (4, T2, NP), (4, A2, NP), (2, T1, NP), (2, A1, NP),
                  (6, HL1, NP), (6, A3, NP), (9, HL3, NP)]
        NSTEP = max(k for k, _, _ in STAGES) + 1

        def emit_body():
            for v in range(NP + NSTEP - 1):
                for k, fn, pmax in STAGES:
                    p = v - k
                    if 0 <= p < pmax:
                        fn(p)

        if loop_n == 1:
            emit_body()
        else:
            with tc.For_i(0, loop_n, 1):
                emit_body()
    return nc


_PROGRAM = None
last_results = None


def _get_program():
    global _PROGRAM
    if _PROGRAM is None:
        _PROGRAM = _build_program()
    return _PROGRAM


def make_in_maps(t, x, dW1, db1, dW2, db2,
                 hw1, htw1, hb1, hw2, htw2, hb2, hw3, htw3, hb3):
    """Host-side sharding/packing. Returns (in_maps, lidx_all, order)."""
    t = np.asarray(t, np.float32)
    x = np.asarray(x, np.float32)

    # --- bin + stable sort (binning identical to the reference) ---
    bins = np.clip(np.floor(t * np.float32(NH)).astype(np.int32), 0, NH - 1)
    order = np.argsort(bins, kind="stable")
    t_s = t[order]
    x_s = x[order]
    bins_s = bins[order]

    # --- static trunk weights with t/ones identity channels ---
    w1a = np.zeros((KX, HA), np.float32)
    w1a[0:D, 0:H] = dW1
    w1a[D, H] = 1.0                      # t passthrough
    b1a = np.zeros((HA, 1), np.float32)
    b1a[0:H, 0] = db1
    b1a[H + 1, 0] = 1.0                  # ones channel born from the bias
    w2a = np.zeros((KP, HA), np.float32)
    w2a[0:H, 0:H] = dW2
    w2a[H, H] = 1.0                      # t passthrough
    w2a[H + 1, 0:H] = db2                # bias via ones channel
    w2a[H + 1, H + 1] = 1.0              # ones passthrough

    # --- per-bin head weight blocks [HA, WTC] = H1(66) | H2(66) | H3(1) ---
    WQ = np.zeros((NH, HA, WTC), np.float32)
    for q in range(NH):
        for li, (hw, htw, hb) in enumerate(
                ((hw1, htw1, hb1), (hw2, htw2, hb2))):
            c = HA * li
            WQ[q, 0:H, c:c + H] = hw[q]
            WQ[q, H, c:c + H] = htw[q]
            WQ[q, H + 1, c:c + H] = hb[q]
            WQ[q, H, c + H] = 1.0        # t passthrough
            WQ[q, H + 1, c + H + 1] = 1.0  # ones passthrough
        WQ[q, 0:H, 2 * HA] = hw3[q][:, 0]
        WQ[q, H, 2 * HA] = htw3[q, 0]
        WQ[q, H + 1, 2 * HA] = hb3[q, 0]

    # --- per-core padded tiling (each 512-row tile single-bin) ---
    in_maps = []
    lidx_all = []
    for c in range(NCORES):
        s = c * RPC
        tb = bins_s[s:s + RPC]
        parts = []
        tile_bins = []
        for q in range(NH):
            sel = np.nonzero(tb == q)[0].astype(np.int64)
            if len(sel) == 0:
                continue
            npad = (-len(sel)) % TILE
            parts.append(np.concatenate([sel, np.full(npad, -1, np.int64)]))
            tile_bins += [q] * ((len(sel) + npad) // TILE)
        lidx = np.concatenate(parts)
        rem = RPAD - len(lidx)
        assert rem >= 0 and rem % TILE == 0
        lidx = np.concatenate([lidx, np.full(rem, -1, np.int64)])
        tile_bins += [0] * (rem // TILE)
        tile_bins = np.asarray(tile_bins, np.int64)
        lidx_all.append(lidx)

        safe = np.where(lidx >= 0, lidx, 0)
        feat = x_s[s:s + RPC][safe]
        tval = t_s[s:s + RPC][safe]
        feat[lidx < 0] = 0.0
        tval[lidx < 0] = 0.0
        xt = np.empty((NG, KX, GT), np.float32)
        xt[:, 0:D, :] = feat.reshape(NG, GT, D).transpose(0, 2, 1)
        xt[:, D, :] = tval.reshape(NG, GT)

        wg = WQ[tile_bins].reshape(NG, 4, HA, WTC).transpose(
            0, 2, 1, 3).reshape(NG, HA, 2 * WGC)

        in_maps.append({
            "xt": xt.astype(BF16), "wg": np.ascontiguousarray(wg).astype(BF16),
            "w1": w1a.astype(BF16), "w2": w2a.astype(BF16), "b1": b1a,
        })
    return in_maps, lidx_all, order


def postprocess(core_outs, lidx_all, order):
    """core_outs: list of per-core 'out' arrays [NT, TILE] -> full [N, 1]."""
    out_sorted = np.empty(N, np.float32)
    for c in range(NCORES):
        flat = np.asarray(core_outs[c], np.float32).reshape(RPAD)
        lidx = lidx_all[c]
        valid = lidx >= 0
        seg = np.empty(RPC, np.float32)
        seg[lidx[valid]] = flat[valid]
        out_sorted[c * RPC:(c + 1) * RPC] = seg
    out = np.empty(N, np.float32)
    out[order] = out_sorted
    return out[:, None]


def kernel(t, x, dW1, db1, dW2, db2,
           hw1, htw1, hb1, hw2, htw2, hb2, hw3, htw3, hb3):
    in_maps, lidx_all, order = make_in_maps(
        t, x, dW1, db1, dW2, db2,
        hw1, htw1, hb1, hw2, htw2, hb2, hw3, htw3, hb3)
    nc = _get_program()
    res = run_bass_kernel_spmd(nc, in_maps, list(range(NCORES)))
    global last_results
    last_results = res
    return postprocess([res.results[c]["out"] for c in range(NCORES)],
                       lidx_all, order)
